# revision 1
# baseline (speedup 1.0000x reference)
"""Trainium2 Bass kernel for MultiHead GQA attention (B=1, S=2048, D=1024,
16 q-heads / 4 kv-heads, GQA group 4, RoPE, causal).

Sharding: tensor-parallel over heads. Core c (of 8) computes 2 query heads
{g, g+4} (c even) or {g+8, g+12} (c odd) with g = c//2, which all attend kv
head g (jnp.tile GQA semantics: q-head h uses kv head h % 4). Wq/Wk/Wv are
column-sharded, Wo row-sharded; each core produces a partial [D, S] output
(transposed) and the host reduces the 8 partials, transposes, and adds bo.

Device dataflow (per core, everything in "transposed" [feature, seq] layout
so no on-device transposes of activations are needed):
  qhT[128,S]  = Wq_c.T @ qT   (+bq)   -> RoPE (block-deinterleaved pairs)
  khT[64,S]   = Wk_c.T @ kT   (+bk)   -> RoPE
  vhT[64,S]   = Wv_c.T @ vT   (+bv)   -> PE-transposed to vh[S,64] (+ ones col)
  scoresT[j,i] = khT[:,j].T @ qhT[:,i]          (PE, K=64)
  pT = exp(scoresT/8)       (ACT, causal-masked via precomputed 0/1 tiles)
  o_aug[65,i] += vh_aug[j].T @ pT[j,i]          (PE; row 64 = softmax denom)
  norm: recip of denom (DVE) -> PE broadcast over 64 partitions -> DVE mul
  outT_partial[e,s] = Wo_c[:,e].T @ attnT       (PE)

RoPE trick: the head_dim is permuted on the host (even dims then odd dims)
in Wq/Wk columns, so rotation pairs are partition blocks [0:32)/[32:64) and
the device applies rope with quadrant-aligned copies + two muls + add using
host-precomputed cos / sign-folded sin tables. The permutation cancels in
q.k dot products and does not touch V or Wo.

The causal mask input is not transferred: the reference mask is tril(ones)
and masking is applied structurally (skipped tiles + 4 precomputed diagonal
mask tiles).
"""

import numpy as np
from contextlib import ExitStack

import concourse.bass as bass
from concourse import bacc
import concourse.mybir as mybir
import concourse.tile as tile
from concourse.bass_utils import run_bass_kernel_spmd

f32 = mybir.dt.float32
f32r = mybir.dt.float32r
USE_F32R = False
MDT = f32r if USE_F32R else f32

S = 2048
D = 1024
HEADS = 16
HD = 64
KVH = 4
N_CORES = 8

ST = 512          # i-tile (free dim of most matmuls)
NS = S // ST      # 4
FP = 128          # contraction chunk
NF = D // FP      # 8
JTS = 128         # j-chunk (key positions per score tile partition dim)
NJ = S // JTS     # 16
NE = D // 128     # 8 output-feature chunks

_CACHE = {}


def _build_program():
    if "nc" in _CACHE:
        return _CACHE["nc"]

    nc = bacc.Bacc("TRN2", target_bir_lowering=False, debug=False)

    def din(name, shape, dt=f32):
        return nc.dram_tensor(name, shape, dt, kind="ExternalInput").ap()

    qT = din("qT", [D, S], MDT)
    kT = din("kT", [D, S], MDT)
    vT = din("vT", [D, S], MDT)
    wq = din("wq", [128, NF * 128], MDT)
    wk = din("wk", [128, NF * 64], MDT)
    wv = din("wv", [128, NF * 64], MDT)
    wo = din("wo", [128, D], MDT)
    bq = din("bq", [128, 1])
    bk = din("bk", [64, 1])
    bv = din("bv", [64, 1])
    cosk = din("cosk", [64, S], MDT)
    sink = din("sink", [64, S], MDT)
    cmask = din("cmask", [128, 896], MDT)
    ident_in = din("ident", [64, 64], MDT)
    ones_in = din("ones", [128, 64], MDT)
    outT = nc.dram_tensor("outT", [D, S], f32, kind="ExternalOutput").ap()
    rcb = nc.dram_tensor("rcb", [2, 2 * ST], f32).ap()   # recip bounce (internal)

    Identity = mybir.ActivationFunctionType.Identity
    Exp = mybir.ActivationFunctionType.Exp
    Ln = mybir.ActivationFunctionType.Ln

    from concourse import library_config

    with tile.TileContext(nc) as tc, ExitStack() as ctx:
        const = ctx.enter_context(tc.tile_pool(name="const", bufs=1))
        big = ctx.enter_context(tc.tile_pool(name="big", bufs=1))
        stream = ctx.enter_context(tc.tile_pool(name="stream", bufs=3))
        ptile = ctx.enter_context(tc.tile_pool(name="ptile", bufs=4))
        small = ctx.enter_context(tc.tile_pool(name="small", bufs=2))
        outb = ctx.enter_context(tc.tile_pool(name="outb", bufs=3))
        psum = ctx.enter_context(tc.tile_pool(name="psum", bufs=4, space="PSUM"))

        def mm(out, lhsT, rhs, start, stop):
            nc.tensor.matmul(out, lhsT=lhsT, rhs=rhs, start=start, stop=stop)

        # ---- weights first (PE needs them first) ----
        wq_sb = const.tile([128, NF, 128], MDT)
        nc.sync.dma_start(out=wq_sb, in_=wq.rearrange("p (nf d) -> p nf d", nf=NF))
        wk_sb = const.tile([128, NF, 64], MDT)
        nc.sync.dma_start(out=wk_sb, in_=wk.rearrange("p (nf d) -> p nf d", nf=NF))
        wv_sb = const.tile([128, NF, 64], MDT)
        nc.sync.dma_start(out=wv_sb, in_=wv.rearrange("p (nf d) -> p nf d", nf=NF))
        # constants used later: issue from DVE queue to keep SP free
        wo_sb = const.tile([128, D], MDT)
        nc.scalar.dma_start(out=wo_sb, in_=wo)
        bq_sb = const.tile([128, 1], f32)
        nc.scalar.dma_start(out=bq_sb, in_=bq)
        bk_sb = const.tile([64, 1], f32)
        nc.scalar.dma_start(out=bk_sb, in_=bk)
        bv_sb = const.tile([64, 1], f32)
        nc.scalar.dma_start(out=bv_sb, in_=bv)
        cosk_sb = const.tile([64, S], MDT)
        nc.scalar.dma_start(out=cosk_sb, in_=cosk)
        sink_sb = const.tile([64, S], MDT)
        nc.scalar.dma_start(out=sink_sb, in_=sink)
        ident = const.tile([64, 64], MDT)
        nc.scalar.dma_start(out=ident, in_=ident_in)
        ones4q = const.tile([128, 64], MDT)
        nc.scalar.dma_start(out=ones4q, in_=ones_in)
        # sliding causal band mask: cm_sb[jp, c] = 1.0 iff jp <= c - 384
        cm_sb = const.tile([128, 896], MDT)
        nc.scalar.dma_start(out=cm_sb, in_=cmask)

        # ---- projections ----
        def project(src_dram, w_sb, nd, copies):
            ps = [psum.tile([128, ST], f32, tag="acc", bufs=4, name=f"pj{s}")
                  for s in range(NS)]
            for f in range(NF):
                xin = stream.tile([128, S], MDT, tag="xin", bufs=5)
                if f == 0:
                    # split so the first s-tile lands (and PE starts) sooner
                    for s in range(NS):
                        nc.sync.dma_start(
                            out=xin[:, s * ST:(s + 1) * ST],
                            in_=src_dram[0:FP, s * ST:(s + 1) * ST])
                else:
                    nc.sync.dma_start(out=xin, in_=src_dram[f * FP:(f + 1) * FP, :])
                for s in range(NS):
                    mm(ps[s][0:nd, :], w_sb[:, f, 0:nd],
                       xin[:, s * ST:(s + 1) * ST],
                       start=(f == 0), stop=(f == NF - 1))
            for s in range(NS):
                for (r0, r1, dst, bias_ap) in copies:
                    nc.scalar.activation(
                        out=dst[:, s * ST:(s + 1) * ST], in_=ps[s][r0:r1, :],
                        func=Identity, bias=bias_ap,
                    )

        qh0 = big.tile([64, S], MDT)
        qh1 = big.tile([64, S], MDT)
        khT = big.tile([64, S], MDT)
        vhT = big.tile([64, S], MDT)
        project(qT, wq_sb, 128,
                [(0, 64, qh0, bq_sb[0:64, :]), (64, 128, qh1, bq_sb[64:128, :])])
        project(kT, wk_sb, 64, [(0, 64, khT, bk_sb)])
        project(vT, wv_sb, 64, [(0, 64, vhT, bv_sb)])

        # ---- RoPE (in-place; pairs are partition blocks [0:32)/[32:64)) ----
        def rope64(x, nm):
            swap = stream.tile([64, S], MDT, tag="swap", name=f"swap_{nm}", bufs=2)
            for (srcp, dstp) in ((32, 0), (0, 32)):
                nc.vector.tensor_copy(swap[dstp:dstp + 32, :], x[srcp:srcp + 32, :])
            nc.vector.tensor_mul(x, x, cosk_sb)
            nc.vector.tensor_mul(swap, swap, sink_sb)
            nc.vector.tensor_add(x, x, swap)

        rope64(khT, "k")
        rope64(qh0, "q0")
        rope64(qh1, "q1")

        # ---- transpose V to [seq, dim] (+ ones column for softmax denom) ----
        vh_aug = big.tile([128, NJ, 65], MDT)
        nc.scalar.copy(vh_aug[:, :, 64], ones4q[0:128, 0:NJ])
        with nc.allow_low_precision(reason="transpose psum, same width as f32"):
            for jt in range(NJ):
                tp = psum.tile([128, 64], MDT, tag="mm", bufs=4, name="tp")
                nc.tensor.transpose(tp, vhT[:, jt * JTS:(jt + 1) * JTS], ident)
                nc.scalar.copy(vh_aug[:, jt, 0:64], tp)

        # ---- attention (2 heads share khT / vh_aug) ----
        attn = big.tile([128, S], MDT)
        for h in range(2):
            q_sl = (qh0, qh1)[h]
            po = [psum.tile([65, ST], f32, tag="acc", bufs=4, name=f"po{s}")
                  for s in range(NS)]

            def finish_pair(p):
                # its (2p, 2p+1) accumulators are complete: normalize + write
                its = (2 * p, 2 * p + 1)
                sums = small.tile([1, 2 * ST], f32, tag="sums", name=f"sums{h}{p}")
                rc = small.tile([1, 2 * ST], f32, tag="rc", name=f"rc{h}{p}")
                lns = small.tile([1, 2 * ST], f32, tag="lns", name=f"lns{h}{p}")
                for it in its:
                    nc.scalar.copy(sums[0:1, ST * (it % 2):ST * (it % 2) + ST],
                                   po[it][64:65, :])
                nc.scalar.activation(out=lns, in_=sums, func=Ln)
                nc.scalar.activation(out=rc, in_=lns, func=Exp, scale=-1.0)
                nc.sync.dma_start(out=rcb[h:h + 1, :], in_=rc)
                for it in its:
                    a_sl = attn[h * 64:(h + 1) * 64, it * ST:(it + 1) * ST]
                    nc.scalar.copy(a_sl, po[it][0:64, :])
                    bct = ptile.tile([128, ST], f32, tag="bct", bufs=2, name="bct")
                    bslc = bct[h * 64:(h + 1) * 64, :]
                    rsrc = rcb[h:h + 1, ST * (it % 2):ST * (it % 2) + ST]
                    rsrc = bass.AP(tensor=rsrc.tensor, offset=rsrc.offset,
                                   ap=[[0, 64]] + list(rsrc.ap)[1:])
                    nc.sync.dma_start(out=bslc, in_=rsrc)
                    nc.vector.tensor_mul(a_sl, a_sl, bslc)

            for jt in range(NJ):
                it0 = jt // 4
                for it in range(it0, NS):
                    # on the diagonal i-tile, columns below the diagonal are
                    # fully masked: skip them in scores/exp/PV entirely
                    lo = (jt - 4 * it) * JTS if it == it0 else 0
                    ps = psum.tile([128, ST], f32, tag="mm", bufs=4)
                    mm(ps[:, lo:], khT[:, jt * JTS:(jt + 1) * JTS],
                       q_sl[:, it * ST + lo:(it + 1) * ST], start=True, stop=True)
                    pt = ptile.tile([128, ST], MDT, tag="pt", bufs=6)
                    nc.scalar.activation(out=pt[:, lo:], in_=ps[:, lo:],
                                         func=Exp, scale=0.125)
                    if it == it0:
                        # partial band: keep iff jp <= (i_off - lo)
                        nc.vector.tensor_mul(pt[:, lo:lo + JTS],
                                             pt[:, lo:lo + JTS],
                                             cm_sb[:, 384:512])
                    mm(po[it][:, lo:], vh_aug[:, jt, :], pt[:, lo:],
                       start=(jt == 0), stop=(jt == 4 * it + 3))
                if jt == 7:
                    finish_pair(0)
            finish_pair(1)

        # ---- output projection (partial over this core's 128 dims) ----
        for it in range(NS):
            for e in range(NE):
                pw = psum.tile([128, ST], f32, tag="mm", bufs=4, name="pw")
                mm(pw, wo_sb[:, e * 128:(e + 1) * 128],
                   attn[:, it * ST:(it + 1) * ST], start=True, stop=True)
                ob = outb.tile([128, ST], f32, tag="ob")
                nc.vector.tensor_copy(ob, pw)
                nc.gpsimd.dma_start(
                    out=outT[e * 128:(e + 1) * 128, it * ST:(it + 1) * ST], in_=ob
                )

    nc.compile()
    _CACHE["nc"] = nc
    return nc


def _host_tables():
    if "tables" in _CACHE:
        return _CACHE["tables"]
    # faithful to reference: exp = -2*arange(0,64,2)/64
    expv = -2.0 * np.arange(0, HD, 2, dtype=np.float32) / HD
    thetas = np.power(np.float32(10000.0), expv).astype(np.float32)    # [32]
    m = np.arange(S, dtype=np.float32)
    freq = np.outer(m, thetas).astype(np.float32)                      # [S, 32]
    cos = np.cos(freq).astype(np.float32).T                            # [32, S]
    sin = np.sin(freq).astype(np.float32).T
    cos64 = np.concatenate([cos, cos], 0)                              # [64, S]
    sin64 = np.concatenate([-sin, sin], 0)                             # [64, S]
    cos64 = np.ascontiguousarray(cos64)
    sin64 = np.ascontiguousarray(sin64)
    perm = np.concatenate([np.arange(0, HD, 2), np.arange(1, HD, 2)])  # deinterleave
    slide = (np.arange(128)[:, None] <= (np.arange(896)[None, :] - 384))
    slide = np.ascontiguousarray(slide.astype(np.float32))
    _CACHE["tables"] = (cos64, sin64, perm, slide)
    return _CACHE["tables"]


def kernel(**inputs):
    q = np.asarray(inputs["q"], np.float32)[0]       # [S, D]
    k = np.asarray(inputs["k"], np.float32)[0]
    v = np.asarray(inputs["v"], np.float32)[0]
    Wq = np.asarray(inputs["Wq"], np.float32)
    Wk = np.asarray(inputs["Wk"], np.float32)
    Wv = np.asarray(inputs["Wv"], np.float32)
    Wo = np.asarray(inputs["Wo"], np.float32)
    bq = np.asarray(inputs["bq"], np.float32)
    bk = np.asarray(inputs["bk"], np.float32)
    bv = np.asarray(inputs["bv"], np.float32)
    bo = np.asarray(inputs["bo"], np.float32)

    cos64, sin64, perm, slide = _host_tables()

    # head_dim deinterleave permutation applied to q/k projection columns
    permQ = np.concatenate([h * HD + perm for h in range(HEADS)])
    permK = np.concatenate([g * HD + perm for g in range(KVH)])
    Wqp = Wq[:, permQ]
    bqp = bq[permQ]
    Wkp = Wk[:, permK]
    bkp = bk[permK]

    qT = np.ascontiguousarray(q.T)
    kT = np.ascontiguousarray(k.T)
    vT = np.ascontiguousarray(v.T)

    in_maps = []
    head_pairs = []
    for c in range(N_CORES):
        g = c // 2
        if c % 2 == 0:
            h0, h1 = g, g + 4
        else:
            h0, h1 = g + 8, g + 12
        head_pairs.append((h0, h1))
        wq_c = np.ascontiguousarray(
            np.concatenate([Wqp[:, h0 * HD:(h0 + 1) * HD],
                            Wqp[:, h1 * HD:(h1 + 1) * HD]], axis=1))
        bq_c = np.ascontiguousarray(
            np.concatenate([bqp[h0 * HD:(h0 + 1) * HD],
                            bqp[h1 * HD:(h1 + 1) * HD]]).reshape(128, 1))
        wo_c = np.ascontiguousarray(
            np.concatenate([Wo[h0 * HD:(h0 + 1) * HD, :],
                            Wo[h1 * HD:(h1 + 1) * HD, :]], axis=0))
        def warr(w):
            # [1024, nd] -> [128, NF*nd] with chunk-of-128-rows as middle dim
            nd = w.shape[1]
            return np.ascontiguousarray(
                w.reshape(NF, FP, nd).transpose(1, 0, 2).reshape(FP, NF * nd))

        in_maps.append({
            "qT": qT, "kT": kT, "vT": vT,
            "wq": warr(wq_c),
            "wk": warr(Wkp[:, g * HD:(g + 1) * HD]),
            "wv": warr(Wv[:, g * HD:(g + 1) * HD]),
            "wo": wo_c,
            "bq": bq_c,
            "bk": np.ascontiguousarray(bkp[g * HD:(g + 1) * HD].reshape(64, 1)),
            "bv": np.ascontiguousarray(bv[g * HD:(g + 1) * HD].reshape(64, 1)),
            "cosk": cos64, "sink": sin64, "cmask": slide,
            "ident": np.eye(64, dtype=np.float32),
            "ones": np.ones((128, 64), np.float32),
        })

    nc = _build_program()
    res = run_bass_kernel_spmd(nc, in_maps, list(range(N_CORES)))
    acc = np.zeros((D, S), np.float32)
    for r in res.results:
        acc += np.asarray(r["outT"], np.float32)
    out = acc.T + bo[None, :]
    return out[None].astype(np.float32)



# revision 9
# speedup vs baseline: 1.7799x; 1.7799x over previous
"""Trainium2 Bass kernel for MultiHead GQA attention (B=1, S=2048, D=1024,
16 q-heads / 4 kv-heads, GQA group 4, RoPE, causal).  bf16 compute, f32 PSUM.

Sharding: tensor-parallel over heads. Core c (of 8) computes 2 query heads
{g, g+4} (c even) or {g+8, g+12} (c odd) with g = c//2, which both attend kv
head g (jnp.tile GQA semantics: q-head h uses kv head h % 4). Wq/Wk/Wv are
column-sharded, Wo row-sharded; each core produces a partial [D, S] bf16
output (transposed) and the host reduces the 8 partials and adds bo.

Per-core dataflow (activations kept in "transposed" [feature, seq] layout):
  - V and K projections run as one col-tiled matmul pass per (f, s):
    V -> PE col groups 0-1 (psum rows 0:64), K -> groups 2-3 (rows 64:128).
  - qh [128, S] = both heads stacked; khT2 [128, S] = kv head duplicated
    into both partition halves (rope on [64:128], then SBUF->SBUF DMA dup).
  - RoPE uses host-deinterleaved head_dim (evens then odds as partition
    blocks), so it is quadrant copies + 2 muls + 1 add on DVE.
  - scores: per (it, jt) the two heads run as row-tiled concurrent matmuls
    (tile_position (0,0) / (64,0)) into a 2-bank psum pair; one ACT exp
    (scale=1/8) emits bf16 probabilities for both heads.
  - causal mask applied structurally: sub-diagonal tiles skipped, diagonal
    128-chunks masked with a [128,128] tril tile on GpSimd.
  - PV accumulates [65, 512] per head (row 64 = softmax denominator via the
    ones column in vh_aug); DVE reciprocal_approx_fast of the denominator,
    DRAM-bounce broadcast, DVE multiply psum->attn bf16.
  - Output projection (row-shard of Wo) deferred one it-tile to hide the
    bounce round-trip; psum pairs drained by DVE/ACT to bf16, DMA'd out.
"""

import numpy as np
import ml_dtypes
from contextlib import ExitStack

import concourse.bass as bass
from concourse import bacc
import concourse.mybir as mybir
import concourse.tile as tile
from concourse.bass_utils import run_bass_kernel_spmd

f32 = mybir.dt.float32
bf16 = mybir.dt.bfloat16
MDT = bf16
NPBF = ml_dtypes.bfloat16

S = 2048
D = 1024
HEADS = 16
HD = 64
KVH = 4
N_CORES = 8

ST = 512          # i-tile (free dim of most matmuls)
NS = S // ST      # 4
FP = 128          # contraction chunk
NF = D // FP      # 8
JTS = 128         # j-chunk (key positions per score tile partition dim)
NJ = S // JTS     # 16
NE = D // 128     # 8 output-feature chunks

_CACHE = {}


def _build_program(debug=False):
    key = ("nc", debug)
    if key in _CACHE:
        return _CACHE[key]

    nc = bacc.Bacc("TRN2", target_bir_lowering=False, debug=False)

    def din(name, shape, dt=MDT):
        return nc.dram_tensor(name, shape, dt, kind="ExternalInput").ap()

    qT = din("qT", [D, S])
    kT = din("kT", [D, S])
    vT = din("vT", [D, S])
    wq = din("wq", [128, NF * 128])
    wk = din("wk", [128, NF * 64])
    wv = din("wv", [128, NF * 64])
    wo = din("wo", [128, D])
    bq = din("bq", [128, 1], f32)
    bkv = din("bkv", [128, 1], f32)       # rows 0:64 = bv, 64:128 = bk
    cosk = din("cosk", [128, S])
    sink = din("sink", [128, S])
    tril = din("tril", [128, 128])
    ident_in = din("ident", [64, 64])
    outT = nc.dram_tensor("outT", [D, S], MDT, kind="ExternalOutput").ap()
    rcb = nc.dram_tensor("rcb", [2, S], MDT).ap()   # recip bounce (internal)

    Exp = mybir.ActivationFunctionType.Exp

    with tile.TileContext(nc) as tc, ExitStack() as ctx, \
            nc.allow_low_precision(reason="bf16 kernel by design"):
        const = ctx.enter_context(tc.tile_pool(name="const", bufs=1))
        big = ctx.enter_context(tc.tile_pool(name="big", bufs=1))
        stream = ctx.enter_context(tc.tile_pool(name="stream", bufs=1))
        ptile = ctx.enter_context(tc.tile_pool(name="ptile", bufs=1))
        small = ctx.enter_context(tc.tile_pool(name="small", bufs=1))
        outb = ctx.enter_context(tc.tile_pool(name="outb", bufs=1))
        psum = ctx.enter_context(tc.tile_pool(name="psum", bufs=1, space="PSUM"))

        def mm(out, lhsT, rhs, start, stop, tp=None, skip=False):
            nc.tensor.matmul(out, lhsT=lhsT, rhs=rhs, start=start, stop=stop,
                             tile_position=tp, skip_group_check=skip)

        # ---- constants: sync ring gets weights+biases (needed first), the
        # scalar ring carries the rest in parallel with the first inputs ----
        wv_sb = const.tile([128, NF, 64], MDT)
        nc.sync.dma_start(out=wv_sb, in_=wv.rearrange("p (f d) -> p f d", f=NF))
        wk_sb = const.tile([128, NF, 64], MDT)
        nc.sync.dma_start(out=wk_sb, in_=wk.rearrange("p (f d) -> p f d", f=NF))
        wq_sb = const.tile([128, NF, 128], MDT)
        nc.sync.dma_start(out=wq_sb, in_=wq.rearrange("p (f d) -> p f d", f=NF))
        bq_sb = const.tile([128, 1], f32)
        nc.sync.dma_start(out=bq_sb, in_=bq)
        bkv_sb = const.tile([128, 1], f32)
        nc.sync.dma_start(out=bkv_sb, in_=bkv)
        cos_sb = const.tile([128, S], MDT)
        nc.scalar.dma_start(out=cos_sb, in_=cosk)
        sin_sb = const.tile([128, S], MDT)
        nc.scalar.dma_start(out=sin_sb, in_=sink)
        tril_sb = const.tile([128, 128], MDT)
        nc.scalar.dma_start(out=tril_sb, in_=tril)
        ident = const.tile([64, 64], MDT)
        nc.scalar.dma_start(out=ident, in_=ident_in)
        wo_sb = const.tile([128, D], MDT)
        nc.scalar.dma_start(out=wo_sb, in_=wo)

        qh = big.tile([128, S], MDT)
        khT2 = big.tile([128, S], MDT)
        vhT = big.tile([64, S], MDT)
        vh_aug = big.tile([128, NJ, 65], MDT)
        attn = big.tile([128, S], MDT)
        nc.vector.memset(vh_aug[:, :, 64], 1.0)

        def oproj(it):
            isl = slice(it * ST, (it + 1) * ST)
            for ep in range(NE // 2):
                pw = psum.tile([128, 2, ST], f32, tag="mm", bufs=2, name="pw")
                for half in range(2):
                    e = 2 * ep + half
                    mm(pw[:, half, :], wo_sb[:, e * 128:(e + 1) * 128],
                       attn[:, isl], start=True, stop=True)
                ob = outb.tile([128, 2, ST], MDT, tag="ob", bufs=3)
                if ep == 3:
                    nc.scalar.copy(ob, pw)
                else:
                    nc.vector.tensor_copy(ob, pw)
                nc.gpsimd.dma_start(
                    out=outT.rearrange("(g p) s -> p g s", p=128)
                    [:, 2 * ep:2 * ep + 2, isl],
                    in_=ob)

        for s in range(NS):
            ssl = slice(s * ST, (s + 1) * ST)
            # ---- input streams for this s-tile (f is a free dim: 1 DMA) ----
            xk = stream.tile([128, NF, ST], MDT, tag="xk", bufs=3)
            nc.sync.dma_start(
                out=xk, in_=kT.rearrange("(f p) s -> p f s", p=128)[:, :, ssl])
            xv = stream.tile([128, NF, ST], MDT, tag="xv", bufs=3)
            nc.sync.dma_start(
                out=xv, in_=vT.rearrange("(f p) s -> p f s", p=128)[:, :, ssl])
            xq = stream.tile([128, NF, ST], MDT, tag="xq", bufs=3)
            nc.sync.dma_start(
                out=xq, in_=qT.rearrange("(f p) s -> p f s", p=128)[:, :, ssl])

            # ---- V (col grp 0-1) + K (col grp 2-3) projection, col-tiled ----
            ps_kv = psum.tile([128, ST], f32, tag="acc", bufs=4, name="pskv")
            for f in range(NF):
                mm(ps_kv[0:64, :], wv_sb[:, f, :], xv[:, f, :],
                   start=(f == 0), stop=(f == NF - 1), tp=(0, 0))
                mm(ps_kv[64:128, :], wk_sb[:, f, :], xk[:, f, :],
                   start=(f == 0), stop=(f == NF - 1), tp=(0, 64), skip=True)
            nc.vector.tensor_scalar_add(vhT[:, ssl], ps_kv[0:64, :],
                                        bkv_sb[0:64, :])
            nc.vector.tensor_scalar_add(khT2[64:128, ssl], ps_kv[64:128, :],
                                        bkv_sb[64:128, :])

            # ---- Q projection (both heads: 128 out dims) ----
            ps_q = psum.tile([128, ST], f32, tag="acc", bufs=4, name="psq")
            for f in range(NF):
                mm(ps_q, wq_sb[:, f, :], xq[:, f, :],
                   start=(f == 0), stop=(f == NF - 1))
            nc.vector.tensor_scalar_add(qh[:, ssl], ps_q, bq_sb)

            # ---- RoPE (pairs are 32-partition blocks; swap + 2 mul + add) ----
            ksw = ptile.tile([128, ST], MDT, tag="ksw", bufs=2)
            nc.vector.tensor_copy(ksw[64:96, :], khT2[96:128, ssl])
            nc.vector.tensor_copy(ksw[96:128, :], khT2[64:96, ssl])
            nc.vector.tensor_mul(ksw[64:128, :], ksw[64:128, :],
                                 sin_sb[64:128, ssl])
            nc.vector.tensor_mul(khT2[64:128, ssl], khT2[64:128, ssl],
                                 cos_sb[64:128, ssl])
            nc.vector.tensor_add(khT2[64:128, ssl], khT2[64:128, ssl],
                                 ksw[64:128, :])
            # duplicate roped kv head into partitions 0:64 for head-0 scores
            nc.gpsimd.dma_start(out=khT2[0:64, ssl], in_=khT2[64:128, ssl])

            qsw = ptile.tile([128, ST], MDT, tag="qsw", bufs=2)
            for (dstp, srcp) in ((0, 32), (32, 0), (64, 96), (96, 64)):
                nc.vector.tensor_copy(qsw[dstp:dstp + 32, :],
                                      qh[srcp:srcp + 32, ssl])
            nc.vector.tensor_mul(qsw, qsw, sin_sb[:, ssl])
            nc.vector.tensor_mul(qh[:, ssl], qh[:, ssl], cos_sb[:, ssl])
            nc.vector.tensor_add(qh[:, ssl], qh[:, ssl], qsw)

            # ---- transpose V to [seq, dim] in vh_aug (ones col = denom) ----
            for m in range(4):
                jt = 4 * s + m
                tp_ps = psum.tile([128, 2, ST], MDT, tag="mm", bufs=2, name="tp")
                nc.tensor.transpose(tp_ps[:, 0, 0:64],
                                    vhT[:, jt * JTS:(jt + 1) * JTS], ident)
                nc.vector.tensor_copy(vh_aug[:, jt, 0:64], tp_ps[:, 0, 0:64])

            # ---- deferred output projection for previous it ----
            if s >= 1:
                oproj(s - 1)

            # ---- attention for it = s (both heads, row-tiled scores) ----
            it = s
            po0 = psum.tile([65, ST], f32, tag="acc", bufs=4, name="po0")
            po1 = psum.tile([65, ST], f32, tag="acc", bufs=4, name="po1")
            jmax = 4 * it + 3
            for jt in range(jmax + 1):
                lo = (jt - 4 * it) * JTS if jt >= 4 * it else 0
                jsl = slice(jt * JTS, (jt + 1) * JTS)
                isl = slice(it * ST + lo, (it + 1) * ST)
                pair = psum.tile([128, 2, ST], f32, tag="mm", bufs=2,
                                 name="pair")
                mm(pair[:, 0, lo:], khT2[0:64, jsl], qh[0:64, isl],
                   start=True, stop=True, tp=(0, 0))
                mm(pair[:, 1, lo:], khT2[64:128, jsl], qh[64:128, isl],
                   start=True, stop=True, tp=(64, 0))
                pt = ptile.tile([128, 2, ST], MDT, tag="pt", bufs=3)
                nc.scalar.activation(out=pt[:, :, lo:], in_=pair[:, :, lo:],
                                     func=Exp, scale=0.125)
                if jt >= 4 * it:
                    for half in range(2):
                        nc.gpsimd.tensor_mul(pt[:, half, lo:lo + JTS],
                                             pt[:, half, lo:lo + JTS], tril_sb)
                if debug and it == 1 and jt == 2:
                    dpt = nc.dram_tensor("d_pt", [128, 2 * ST], MDT,
                                         kind="ExternalOutput").ap()
                    nc.sync.dma_start(
                        out=dpt.rearrange("p (a b) -> p a b", a=2), in_=pt)
                mm(po0[:, lo:], vh_aug[:, jt, :], pt[:, 0, lo:],
                   start=(jt == 0), stop=(jt == jmax))
                mm(po1[:, lo:], vh_aug[:, jt, :], pt[:, 1, lo:],
                   start=(jt == 0), stop=(jt == jmax))

            # ---- softmax denominators -> bf16 reciprocals -> bounce ----
            isl = slice(it * ST, (it + 1) * ST)
            sums = small.tile([1, 2, ST], f32, tag="sums", bufs=2)
            rc = small.tile([1, 2, ST], f32, tag="rc", bufs=2)
            rcb16 = small.tile([1, 2, ST], MDT, tag="rcb16", bufs=2)
            nc.scalar.copy(sums[:, 0, :], po0[64:65, :])
            nc.scalar.copy(sums[:, 1, :], po1[64:65, :])
            nc.vector.reciprocal_approx_fast(rc, sums)
            nc.vector.tensor_copy(rcb16, rc)
            nc.sync.dma_start(out=rcb[0:1, isl], in_=rcb16[:, 0, :])
            nc.sync.dma_start(out=rcb[1:2, isl], in_=rcb16[:, 1, :])
            bct = ptile.tile([128, ST], MDT, tag="bct", bufs=2)
            for h in range(2):
                rsrc = rcb[h:h + 1, isl]
                rsrc = bass.AP(tensor=rsrc.tensor, offset=rsrc.offset,
                               ap=[[0, 64]] + list(rsrc.ap)[1:])
                nc.gpsimd.dma_start(out=bct[h * 64:(h + 1) * 64, :], in_=rsrc)
            if debug and it == 1:
                dbc = nc.dram_tensor("d_bct", [128, ST], MDT,
                                     kind="ExternalOutput").ap()
                nc.sync.dma_start(out=dbc, in_=bct)
            nc.vector.tensor_mul(attn[0:64, isl], po0[0:64, :], bct[0:64, :])
            nc.vector.tensor_mul(attn[64:128, isl], po1[0:64, :],
                                 bct[64:128, :])

        oproj(NS - 1)

        if debug:
            dqh = nc.dram_tensor("d_qh", [128, S], MDT,
                                 kind="ExternalOutput").ap()
            dkh = nc.dram_tensor("d_khT2", [128, S], MDT,
                                 kind="ExternalOutput").ap()
            dvh = nc.dram_tensor("d_vhT", [64, S], MDT,
                                 kind="ExternalOutput").ap()
            dva = nc.dram_tensor("d_vh_aug", [128, NJ * 65], MDT,
                                 kind="ExternalOutput").ap()
            dat = nc.dram_tensor("d_attn", [128, S], MDT,
                                 kind="ExternalOutput").ap()
            nc.sync.dma_start(out=dqh, in_=qh)
            nc.sync.dma_start(out=dkh, in_=khT2)
            nc.sync.dma_start(out=dvh, in_=vhT)
            nc.sync.dma_start(
                out=dva.rearrange("p (j e) -> p j e", j=NJ), in_=vh_aug)
            nc.sync.dma_start(out=dat, in_=attn)

    nc.compile()
    _CACHE[key] = nc
    return nc


def _host_tables():
    if "tables" in _CACHE:
        return _CACHE["tables"]
    # faithful to reference: exp = -2*arange(0,64,2)/64
    expv = -2.0 * np.arange(0, HD, 2, dtype=np.float32) / HD
    thetas = np.power(np.float32(10000.0), expv).astype(np.float32)    # [32]
    m = np.arange(S, dtype=np.float32)
    freq = np.outer(m, thetas).astype(np.float32)                      # [S, 32]
    cos = np.cos(freq).astype(np.float32).T                            # [32, S]
    sin = np.sin(freq).astype(np.float32).T
    cos128 = np.concatenate([cos, cos, cos, cos], 0)                   # [128, S]
    sin128 = np.concatenate([-sin, sin, -sin, sin], 0)
    perm = np.concatenate([np.arange(0, HD, 2), np.arange(1, HD, 2)])  # deint
    trilm = (np.arange(128)[:, None] <= np.arange(128)[None, :])
    _CACHE["tables"] = (
        np.ascontiguousarray(cos128.astype(NPBF)),
        np.ascontiguousarray(sin128.astype(NPBF)),
        perm,
        np.ascontiguousarray(trilm.astype(NPBF)),
    )
    return _CACHE["tables"]


def _warr(w):
    # [1024, nd] -> [128, NF*nd] with chunk-of-128-rows as middle dim
    nd = w.shape[1]
    return np.ascontiguousarray(
        w.reshape(NF, FP, nd).transpose(1, 0, 2).reshape(FP, NF * nd)
        .astype(NPBF))


def kernel(**inputs):
    q = np.asarray(inputs["q"], np.float32)[0]       # [S, D]
    k = np.asarray(inputs["k"], np.float32)[0]
    v = np.asarray(inputs["v"], np.float32)[0]
    Wq = np.asarray(inputs["Wq"], np.float32)
    Wk = np.asarray(inputs["Wk"], np.float32)
    Wv = np.asarray(inputs["Wv"], np.float32)
    Wo = np.asarray(inputs["Wo"], np.float32)
    bq = np.asarray(inputs["bq"], np.float32)
    bk = np.asarray(inputs["bk"], np.float32)
    bv = np.asarray(inputs["bv"], np.float32)
    bo = np.asarray(inputs["bo"], np.float32)

    cos128, sin128, perm, trilm = _host_tables()

    # head_dim deinterleave permutation applied to q/k projection columns
    permQ = np.concatenate([h * HD + perm for h in range(HEADS)])
    permK = np.concatenate([g * HD + perm for g in range(KVH)])
    Wqp = Wq[:, permQ]
    bqp = bq[permQ]
    Wkp = Wk[:, permK]
    bkp = bk[permK]

    qT = np.ascontiguousarray(q.T.astype(NPBF))
    kT = np.ascontiguousarray(k.T.astype(NPBF))
    vT = np.ascontiguousarray(v.T.astype(NPBF))
    ident64 = np.eye(64, dtype=np.float32).astype(NPBF)

    in_maps = []
    for c in range(N_CORES):
        g = c // 2
        if c % 2 == 0:
            h0, h1 = g, g + 4
        else:
            h0, h1 = g + 8, g + 12
        wq_c = np.concatenate([Wqp[:, h0 * HD:(h0 + 1) * HD],
                               Wqp[:, h1 * HD:(h1 + 1) * HD]], axis=1)
        bq_c = np.ascontiguousarray(
            np.concatenate([bqp[h0 * HD:(h0 + 1) * HD],
                            bqp[h1 * HD:(h1 + 1) * HD]]).reshape(128, 1))
        bkv_c = np.ascontiguousarray(
            np.concatenate([bv[g * HD:(g + 1) * HD],
                            bkp[g * HD:(g + 1) * HD]]).reshape(128, 1))
        wo_c = np.ascontiguousarray(
            np.concatenate([Wo[h0 * HD:(h0 + 1) * HD, :],
                            Wo[h1 * HD:(h1 + 1) * HD, :]], axis=0)
            .astype(NPBF))

        in_maps.append({
            "qT": qT, "kT": kT, "vT": vT,
            "wq": _warr(wq_c),
            "wk": _warr(Wkp[:, g * HD:(g + 1) * HD]),
            "wv": _warr(Wv[:, g * HD:(g + 1) * HD]),
            "wo": wo_c,
            "bq": bq_c,
            "bkv": bkv_c,
            "cosk": cos128, "sink": sin128, "tril": trilm,
            "ident": ident64,
        })

    nc = _build_program()
    res = run_bass_kernel_spmd(nc, in_maps, list(range(N_CORES)))
    acc = np.zeros((D, S), np.float32)
    for r in res.results:
        acc += np.asarray(r["outT"], np.float32)
    out = acc.T + bo[None, :]
    return out[None].astype(np.float32)


# revision 14
# speedup vs baseline: 1.9398x; 1.0898x over previous
"""Trainium2 Bass kernel for MultiHead GQA attention (B=1, S=2048, D=1024,
16 q-heads / 4 kv-heads, GQA group 4, RoPE, causal).  bf16 compute, f32 PSUM.

Sharding: tensor-parallel over heads. Core c (of 8) computes 2 query heads
{g, g+4} (c even) or {g+8, g+12} (c odd) with g = c//2, which both attend kv
head g (jnp.tile GQA semantics: q-head h uses kv head h % 4). Wq/Wk/Wv are
column-sharded, Wo row-sharded; each core produces a partial [D, S] bf16
output (transposed) and the host reduces the 8 partials and adds bo.

Per-core dataflow (activations kept in "transposed" [feature, seq] layout):
  - V and K projections run as one col-tiled matmul pass per (f, s):
    V -> PE col groups 0-1 (psum rows 0:64), K -> groups 2-3 (rows 64:128).
  - qh [128, S] = both heads stacked; khT2 [128, S] = kv head duplicated
    into both partition halves (rope on [64:128], then SBUF->SBUF DMA dup).
  - RoPE uses host-deinterleaved head_dim (evens then odds as partition
    blocks), so it is quadrant copies + 2 muls + 1 add on DVE.
  - scores: per (it, jt) the two heads run as row-tiled concurrent matmuls
    (tile_position (0,0) / (64,0)) into a 2-bank psum pair; one ACT exp
    (scale=1/8) emits bf16 probabilities for both heads.
  - causal mask applied structurally: sub-diagonal tiles skipped, diagonal
    128-chunks masked with a [128,128] tril tile on GpSimd.
  - PV accumulates [65, 512] per head (row 64 = softmax denominator via the
    ones column in vh_aug); DVE reciprocal_approx_fast of the denominator,
    DRAM-bounce broadcast, DVE multiply psum->attn bf16.
  - Output projection (row-shard of Wo) deferred one it-tile to hide the
    bounce round-trip; psum pairs drained by DVE/ACT to bf16, DMA'd out.
"""

import numpy as np
import ml_dtypes
from contextlib import ExitStack

import concourse.bass as bass
from concourse import bacc
import concourse.mybir as mybir
import concourse.tile as tile
from concourse.bass_utils import run_bass_kernel_spmd

f32 = mybir.dt.float32
bf16 = mybir.dt.bfloat16
MDT = bf16
NPBF = ml_dtypes.bfloat16

S = 2048
D = 1024
HEADS = 16
HD = 64
KVH = 4
N_CORES = 8

ST = 512          # i-tile (free dim of most matmuls)
NS = S // ST      # 4
FP = 128          # contraction chunk
NF = D // FP      # 8
JTS = 128         # j-chunk (key positions per score tile partition dim)
NJ = S // JTS     # 16
NE = D // 128     # 8 output-feature chunks

_CACHE = {}


def _build_program(debug=False):
    key = ("nc", debug)
    if key in _CACHE:
        return _CACHE[key]

    nc = bacc.Bacc("TRN2", target_bir_lowering=False, debug=False)

    def din(name, shape, dt=MDT):
        return nc.dram_tensor(name, shape, dt, kind="ExternalInput").ap()

    # inputs pre-chunked on host: [s-tile][partition][f * 512] contiguous
    qT = din("qT", [NS, 128, NF * ST])
    kT = din("kT", [NS, 128, NF * ST])
    vT = din("vT", [NS, 128, NF * ST])
    wq = din("wq", [128, NF * 128])
    wk = din("wk", [128, NF * 64])
    wv = din("wv", [128, NF * 64])
    wo = din("wo", [128, D])
    bq = din("bq", [128, 1], f32)
    bkv = din("bkv", [128, 1], f32)       # rows 0:64 = bv, 64:128 = bk
    cosk = din("cosk", [128, S])
    sink = din("sink", [128, S])
    tril = din("tril", [128, 128])
    ident_in = din("ident", [64, 64])
    outT = nc.dram_tensor("outT", [D, S], MDT, kind="ExternalOutput").ap()
    rcb = nc.dram_tensor("rcb", [2, S], MDT).ap()   # recip bounce (internal)

    Exp = mybir.ActivationFunctionType.Exp

    with tile.TileContext(nc) as tc, ExitStack() as ctx, \
            nc.allow_low_precision(reason="bf16 kernel by design"):
        const = ctx.enter_context(tc.tile_pool(name="const", bufs=1))
        big = ctx.enter_context(tc.tile_pool(name="big", bufs=1))
        stream = ctx.enter_context(tc.tile_pool(name="stream", bufs=1))
        ptile = ctx.enter_context(tc.tile_pool(name="ptile", bufs=1))
        small = ctx.enter_context(tc.tile_pool(name="small", bufs=1))
        outb = ctx.enter_context(tc.tile_pool(name="outb", bufs=1))
        psum = ctx.enter_context(tc.tile_pool(name="psum", bufs=1, space="PSUM"))

        def mm(out, lhsT, rhs, start, stop, tp=None, skip=False):
            nc.tensor.matmul(out, lhsT=lhsT, rhs=rhs, start=start, stop=stop,
                             tile_position=tp, skip_group_check=skip)

        # ---- constants: sync ring gets V/K weights (needed first); q input
        # chunks ride the gpsimd ring; tables/wo on the scalar ring ----
        wv_sb = const.tile([128, NF, 64], MDT)
        nc.sync.dma_start(out=wv_sb, in_=wv.rearrange("p (f d) -> p f d", f=NF))
        wk_sb = const.tile([128, NF, 64], MDT)
        nc.sync.dma_start(out=wk_sb, in_=wk.rearrange("p (f d) -> p f d", f=NF))
        cos_sb = const.tile([128, S], MDT)
        nc.scalar.dma_start(out=cos_sb, in_=cosk)
        sin_sb = const.tile([128, S], MDT)
        nc.scalar.dma_start(out=sin_sb, in_=sink)
        tril_sb = const.tile([128, 128], MDT)
        nc.scalar.dma_start(out=tril_sb, in_=tril)
        ident = const.tile([64, 64], MDT)
        nc.scalar.dma_start(out=ident, in_=ident_in)
        wo_sb = const.tile([128, D], MDT)
        nc.scalar.dma_start(out=wo_sb, in_=wo)
        wq_sb = const.tile([128, NF, 128], MDT)
        bq_sb = const.tile([128, 1], f32)
        bkv_sb = const.tile([128, 1], f32)

        qh = big.tile([128, S], MDT)
        khT2 = big.tile([128, S], MDT)
        vhT = big.tile([64, S], MDT)
        vh_aug = big.tile([128, NJ, 65], MDT)
        attn = big.tile([128, S], MDT)
        nc.vector.memset(vh_aug[:, :, 64], 1.0)

        def oproj(it):
            isl = slice(it * ST, (it + 1) * ST)
            for ep in range(NE // 2):
                pw = psum.tile([128, 2, ST], f32, tag="mm", bufs=2, name="pw")
                for half in range(2):
                    e = 2 * ep + half
                    mm(pw[:, half, :], wo_sb[:, e * 128:(e + 1) * 128],
                       attn[:, isl], start=True, stop=True)
                ob = outb.tile([128, 2, ST], MDT, tag="ob", bufs=3)
                if ep % 2 == 1:
                    nc.scalar.copy(ob, pw)
                else:
                    nc.vector.tensor_copy(ob, pw)
                nc.gpsimd.dma_start(
                    out=outT.rearrange("(g p) s -> p g s", p=128)
                    [:, 2 * ep:2 * ep + 2, isl],
                    in_=ob)

        def proj_rope(s):
            ssl = slice(s * ST, (s + 1) * ST)
            # ---- input streams for this s-tile (contiguous 1MB chunks) ----
            xv = stream.tile([128, NF, ST], MDT, tag="xv", bufs=3)
            nc.sync.dma_start(
                out=xv, in_=vT[s].rearrange("p (f c) -> p f c", f=NF))
            xk = stream.tile([128, NF, ST], MDT, tag="xk", bufs=3)
            nc.sync.dma_start(
                out=xk, in_=kT[s].rearrange("p (f c) -> p f c", f=NF))
            xq = stream.tile([128, NF, ST], MDT, tag="xq", bufs=3)
            nc.gpsimd.dma_start(
                out=xq, in_=qT[s].rearrange("p (f c) -> p f c", f=NF))
            if s == 0:
                nc.sync.dma_start(
                    out=wq_sb, in_=wq.rearrange("p (f d) -> p f d", f=NF))
                nc.sync.dma_start(out=bq_sb, in_=bq)
                nc.sync.dma_start(out=bkv_sb, in_=bkv)

            # ---- V (col grp 0-1) + K (col grp 2-3) projection, col-tiled ----
            ps_kv = psum.tile([128, ST], f32, tag="acc", bufs=4, name="pskv")
            for f in range(NF):
                mm(ps_kv[0:64, :], wv_sb[:, f, :], xv[:, f, :],
                   start=(f == 0), stop=(f == NF - 1), tp=(0, 0))
                mm(ps_kv[64:128, :], wk_sb[:, f, :], xk[:, f, :],
                   start=(f == 0), stop=(f == NF - 1), tp=(0, 64), skip=True)
            nc.vector.tensor_scalar_add(vhT[:, ssl], ps_kv[0:64, :],
                                        bkv_sb[0:64, :])
            nc.vector.tensor_scalar_add(khT2[64:128, ssl], ps_kv[64:128, :],
                                        bkv_sb[64:128, :])

            # ---- Q projection (both heads: 128 out dims) ----
            ps_q = psum.tile([128, ST], f32, tag="acc", bufs=4, name="psq")
            for f in range(NF):
                mm(ps_q, wq_sb[:, f, :], xq[:, f, :],
                   start=(f == 0), stop=(f == NF - 1))
            nc.vector.tensor_scalar_add(qh[:, ssl], ps_q, bq_sb)

            # ---- RoPE (pairs are 32-partition blocks; swap + 2 mul + add) ----
            ksw = ptile.tile([128, ST], MDT, tag="ksw", bufs=2)
            nc.vector.tensor_copy(ksw[64:96, :], khT2[96:128, ssl])
            nc.vector.tensor_copy(ksw[96:128, :], khT2[64:96, ssl])
            nc.vector.tensor_mul(ksw[64:128, :], ksw[64:128, :],
                                 sin_sb[64:128, ssl])
            nc.vector.tensor_mul(khT2[64:128, ssl], khT2[64:128, ssl],
                                 cos_sb[64:128, ssl])
            nc.vector.tensor_add(khT2[64:128, ssl], khT2[64:128, ssl],
                                 ksw[64:128, :])
            # duplicate roped kv head into partitions 0:64 for head-0 scores
            nc.gpsimd.dma_start(out=khT2[0:64, ssl], in_=khT2[64:128, ssl])

            qsw = ptile.tile([128, ST], MDT, tag="qsw", bufs=2)
            for (dstp, srcp) in ((0, 32), (32, 0), (64, 96), (96, 64)):
                nc.vector.tensor_copy(qsw[dstp:dstp + 32, :],
                                      qh[srcp:srcp + 32, ssl])
            nc.vector.tensor_mul(qsw, qsw, sin_sb[:, ssl])
            nc.vector.tensor_mul(qh[:, ssl], qh[:, ssl], cos_sb[:, ssl])
            nc.vector.tensor_add(qh[:, ssl], qh[:, ssl], qsw)

            # ---- transpose V to [seq, dim] in vh_aug (ones col = denom) ----
            for m in range(4):
                jt = 4 * s + m
                tp_ps = psum.tile([128, 2, ST], MDT, tag="mm", bufs=2, name="tp")
                nc.tensor.transpose(tp_ps[:, 0, 0:64],
                                    vhT[:, jt * JTS:(jt + 1) * JTS], ident)
                nc.vector.tensor_copy(vh_aug[:, jt, 0:64], tp_ps[:, 0, 0:64])

        def attn_block(it):
            # ---- attention for it (both heads, row-tiled scores) ----
            po0 = psum.tile([65, ST], f32, tag="acc", bufs=4, name="po0")
            po1 = psum.tile([65, ST], f32, tag="acc", bufs=4, name="po1")
            jmax = 4 * it + 3
            for jt in range(jmax + 1):
                lo = (jt - 4 * it) * JTS if jt >= 4 * it else 0
                jsl = slice(jt * JTS, (jt + 1) * JTS)
                isl = slice(it * ST + lo, (it + 1) * ST)
                pair = psum.tile([128, 2, ST], f32, tag="mm", bufs=2,
                                 name="pair")
                mm(pair[:, 0, lo:], khT2[0:64, jsl], qh[0:64, isl],
                   start=True, stop=True, tp=(0, 0))
                mm(pair[:, 1, lo:], khT2[64:128, jsl], qh[64:128, isl],
                   start=True, stop=True, tp=(64, 0))
                pt = ptile.tile([128, 2, ST], MDT, tag="pt", bufs=3)
                nc.scalar.activation(out=pt[:, :, lo:], in_=pair[:, :, lo:],
                                     func=Exp, scale=0.125)
                if jt >= 4 * it:
                    for half in range(2):
                        nc.gpsimd.tensor_mul(pt[:, half, lo:lo + JTS],
                                             pt[:, half, lo:lo + JTS], tril_sb)
                if debug and it == 1 and jt == 2:
                    dpt = nc.dram_tensor("d_pt", [128, 2 * ST], MDT,
                                         kind="ExternalOutput").ap()
                    nc.sync.dma_start(
                        out=dpt.rearrange("p (a b) -> p a b", a=2), in_=pt)
                mm(po0[:, lo:], vh_aug[:, jt, :], pt[:, 0, lo:],
                   start=(jt == 0), stop=(jt == jmax))
                mm(po1[:, lo:], vh_aug[:, jt, :], pt[:, 1, lo:],
                   start=(jt == 0), stop=(jt == jmax))

            # ---- softmax denominators -> bf16 reciprocals -> bounce ----
            isl = slice(it * ST, (it + 1) * ST)
            sums = small.tile([1, 2, ST], f32, tag="sums", bufs=2)
            rc = small.tile([1, 2, ST], f32, tag="rc", bufs=2)
            rcb16 = small.tile([1, 2, ST], MDT, tag="rcb16", bufs=2)
            nc.scalar.copy(sums[:, 0, :], po0[64:65, :])
            nc.scalar.copy(sums[:, 1, :], po1[64:65, :])
            nc.vector.reciprocal_approx_fast(rc, sums)
            nc.vector.tensor_copy(rcb16, rc)
            nc.sync.dma_start(out=rcb[0:1, isl], in_=rcb16[:, 0, :])
            nc.sync.dma_start(out=rcb[1:2, isl], in_=rcb16[:, 1, :])
            bct = ptile.tile([128, ST], MDT, tag="bct", bufs=2)
            for h in range(2):
                rsrc = rcb[h:h + 1, isl]
                rsrc = bass.AP(tensor=rsrc.tensor, offset=rsrc.offset,
                               ap=[[0, 64]] + list(rsrc.ap)[1:])
                nc.gpsimd.dma_start(out=bct[h * 64:(h + 1) * 64, :], in_=rsrc)
            if debug and it == 1:
                dbc = nc.dram_tensor("d_bct", [128, ST], MDT,
                                     kind="ExternalOutput").ap()
                nc.sync.dma_start(out=dbc, in_=bct)
            nc.vector.tensor_mul(attn[0:64, isl], po0[0:64, :], bct[0:64, :])
            nc.vector.tensor_mul(attn[64:128, isl], po1[0:64, :],
                                 bct[64:128, :])

        # software pipeline: attention lags projections by one s-tile,
        # output projection lags attention by one more
        for s in range(NS):
            proj_rope(s)
            if s >= 1:
                attn_block(s - 1)
            if s >= 2:
                oproj(s - 2)
        attn_block(NS - 1)
        oproj(NS - 2)
        oproj(NS - 1)

        if debug:
            dqh = nc.dram_tensor("d_qh", [128, S], MDT,
                                 kind="ExternalOutput").ap()
            dkh = nc.dram_tensor("d_khT2", [128, S], MDT,
                                 kind="ExternalOutput").ap()
            dvh = nc.dram_tensor("d_vhT", [64, S], MDT,
                                 kind="ExternalOutput").ap()
            dva = nc.dram_tensor("d_vh_aug", [128, NJ * 65], MDT,
                                 kind="ExternalOutput").ap()
            dat = nc.dram_tensor("d_attn", [128, S], MDT,
                                 kind="ExternalOutput").ap()
            nc.sync.dma_start(out=dqh, in_=qh)
            nc.sync.dma_start(out=dkh, in_=khT2)
            nc.sync.dma_start(out=dvh, in_=vhT)
            nc.sync.dma_start(
                out=dva.rearrange("p (j e) -> p j e", j=NJ), in_=vh_aug)
            nc.sync.dma_start(out=dat, in_=attn)

    nc.compile()
    _CACHE[key] = nc
    return nc


def _host_tables():
    if "tables" in _CACHE:
        return _CACHE["tables"]
    # faithful to reference: exp = -2*arange(0,64,2)/64
    expv = -2.0 * np.arange(0, HD, 2, dtype=np.float32) / HD
    thetas = np.power(np.float32(10000.0), expv).astype(np.float32)    # [32]
    m = np.arange(S, dtype=np.float32)
    freq = np.outer(m, thetas).astype(np.float32)                      # [S, 32]
    cos = np.cos(freq).astype(np.float32).T                            # [32, S]
    sin = np.sin(freq).astype(np.float32).T
    cos128 = np.concatenate([cos, cos, cos, cos], 0)                   # [128, S]
    sin128 = np.concatenate([-sin, sin, -sin, sin], 0)
    perm = np.concatenate([np.arange(0, HD, 2), np.arange(1, HD, 2)])  # deint
    trilm = (np.arange(128)[:, None] <= np.arange(128)[None, :])
    _CACHE["tables"] = (
        np.ascontiguousarray(cos128.astype(NPBF)),
        np.ascontiguousarray(sin128.astype(NPBF)),
        perm,
        np.ascontiguousarray(trilm.astype(NPBF)),
    )
    return _CACHE["tables"]


def _warr(w):
    # [1024, nd] -> [128, NF*nd] with chunk-of-128-rows as middle dim
    nd = w.shape[1]
    return np.ascontiguousarray(
        w.reshape(NF, FP, nd).transpose(1, 0, 2).reshape(FP, NF * nd)
        .astype(NPBF))


def kernel(**inputs):
    q = np.asarray(inputs["q"], np.float32)[0]       # [S, D]
    k = np.asarray(inputs["k"], np.float32)[0]
    v = np.asarray(inputs["v"], np.float32)[0]
    Wq = np.asarray(inputs["Wq"], np.float32)
    Wk = np.asarray(inputs["Wk"], np.float32)
    Wv = np.asarray(inputs["Wv"], np.float32)
    Wo = np.asarray(inputs["Wo"], np.float32)
    bq = np.asarray(inputs["bq"], np.float32)
    bk = np.asarray(inputs["bk"], np.float32)
    bv = np.asarray(inputs["bv"], np.float32)
    bo = np.asarray(inputs["bo"], np.float32)

    cos128, sin128, perm, trilm = _host_tables()

    # head_dim deinterleave permutation applied to q/k projection columns
    permQ = np.concatenate([h * HD + perm for h in range(HEADS)])
    permK = np.concatenate([g * HD + perm for g in range(KVH)])
    Wqp = Wq[:, permQ]
    bqp = bq[permQ]
    Wkp = Wk[:, permK]
    bkp = bk[permK]

    def chunk(x):
        # [S, D] -> [NS, 128, NF*512]: xc[s, p, f*512+c] = x[s*512+c, f*128+p]
        xc = x.T.reshape(NF, FP, NS, ST).transpose(2, 1, 0, 3)
        return np.ascontiguousarray(
            xc.reshape(NS, FP, NF * ST).astype(NPBF))

    qT = chunk(q)
    kT = chunk(k)
    vT = chunk(v)
    ident64 = np.eye(64, dtype=np.float32).astype(NPBF)

    in_maps = []
    for c in range(N_CORES):
        g = c // 2
        if c % 2 == 0:
            h0, h1 = g, g + 4
        else:
            h0, h1 = g + 8, g + 12
        wq_c = np.concatenate([Wqp[:, h0 * HD:(h0 + 1) * HD],
                               Wqp[:, h1 * HD:(h1 + 1) * HD]], axis=1)
        bq_c = np.ascontiguousarray(
            np.concatenate([bqp[h0 * HD:(h0 + 1) * HD],
                            bqp[h1 * HD:(h1 + 1) * HD]]).reshape(128, 1))
        bkv_c = np.ascontiguousarray(
            np.concatenate([bv[g * HD:(g + 1) * HD],
                            bkp[g * HD:(g + 1) * HD]]).reshape(128, 1))
        wo_c = np.ascontiguousarray(
            np.concatenate([Wo[h0 * HD:(h0 + 1) * HD, :],
                            Wo[h1 * HD:(h1 + 1) * HD, :]], axis=0)
            .astype(NPBF))

        in_maps.append({
            "qT": qT, "kT": kT, "vT": vT,
            "wq": _warr(wq_c),
            "wk": _warr(Wkp[:, g * HD:(g + 1) * HD]),
            "wv": _warr(Wv[:, g * HD:(g + 1) * HD]),
            "wo": wo_c,
            "bq": bq_c,
            "bkv": bkv_c,
            "cosk": cos128, "sink": sin128, "tril": trilm,
            "ident": ident64,
        })

    nc = _build_program()
    res = run_bass_kernel_spmd(nc, in_maps, list(range(N_CORES)))
    acc = np.zeros((D, S), np.float32)
    for r in res.results:
        acc += np.asarray(r["outT"], np.float32)
    out = acc.T + bo[None, :]
    return out[None].astype(np.float32)


# revision 19
# speedup vs baseline: 2.0203x; 1.0415x over previous
"""Trainium2 Bass kernel for MultiHead GQA attention (B=1, S=2048, D=1024,
16 q-heads / 4 kv-heads, GQA group 4, RoPE, causal).  bf16 compute, f32 PSUM.

Sharding: tensor-parallel over heads. Core c (of 8) computes 2 query heads
{g, g+4} (c even) or {g+8, g+12} (c odd) with g = c//2, which both attend kv
head g (jnp.tile GQA semantics: q-head h uses kv head h % 4). Wq/Wk/Wv are
column-sharded, Wo row-sharded; each core produces a partial [D, S] bf16
output (transposed) and the host reduces the 8 partials and adds bo.

Per-core dataflow (activations kept in "transposed" [feature, seq] layout):
  - V and K projections run as one col-tiled matmul pass per (f, s):
    V -> PE col groups 0-1 (psum rows 0:64), K -> groups 2-3 (rows 64:128).
  - qh [128, S] = both heads stacked; khT2 [128, S] = kv head duplicated
    into both partition halves (rope on [64:128], then SBUF->SBUF DMA dup).
  - RoPE uses host-deinterleaved head_dim (evens then odds as partition
    blocks), so it is quadrant copies + 2 muls + 1 add on DVE.
  - scores: per (it, jt) the two heads run as row-tiled concurrent matmuls
    (tile_position (0,0) / (64,0)) into a 2-bank psum pair; one ACT exp
    (scale=1/8) emits bf16 probabilities for both heads.
  - causal mask applied structurally: sub-diagonal tiles skipped, diagonal
    128-chunks masked with a [128,128] tril tile on GpSimd.
  - PV accumulates [65, 512] per head (row 64 = softmax denominator via the
    ones column in vh_aug); DVE reciprocal_approx_fast of the denominator,
    DRAM-bounce broadcast, DVE multiply psum->attn bf16.
  - Output projection (row-shard of Wo) deferred one it-tile to hide the
    bounce round-trip; psum pairs drained by DVE/ACT to bf16, DMA'd out.
"""

import numpy as np
import ml_dtypes
from contextlib import ExitStack

import concourse.bass as bass
from concourse import bacc
import concourse.mybir as mybir
import concourse.tile as tile
from concourse.bass_utils import run_bass_kernel_spmd

f32 = mybir.dt.float32
bf16 = mybir.dt.bfloat16
MDT = bf16
NPBF = ml_dtypes.bfloat16

S = 2048
D = 1024
HEADS = 16
HD = 64
KVH = 4
N_CORES = 8

ST = 512          # i-tile (free dim of most matmuls)
NS = S // ST      # 4
FP = 128          # contraction chunk
NF = D // FP      # 8
JTS = 128         # j-chunk (key positions per score tile partition dim)
NJ = S // JTS     # 16
NE = D // 128     # 8 output-feature chunks

_CACHE = {}


def _build_program(debug=False):
    key = ("nc", debug)
    if key in _CACHE:
        return _CACHE[key]

    nc = bacc.Bacc("TRN2", target_bir_lowering=False, debug=False)

    def din(name, shape, dt=MDT):
        return nc.dram_tensor(name, shape, dt, kind="ExternalInput").ap()

    # inputs pre-chunked on host: [s-tile][partition][f * 512] contiguous
    qT = din("qT", [NS, 128, NF * ST])
    kT = din("kT", [NS, 128, NF * ST])
    vT = din("vT", [NS, 128, NF * ST])
    wq = din("wq", [128, NF * 128])
    wk = din("wk", [128, NF * 64])
    wv = din("wv", [128, NF * 64])
    wo = din("wo", [128, D])
    bq = din("bq", [128, 1], f32)
    bkv = din("bkv", [128, 1], f32)       # rows 0:64 = bv, 64:128 = bk
    cosk = din("cosk", [128, S])
    sink = din("sink", [128, S])
    tril = din("tril", [128, 128])
    ident_in = din("ident", [64, 64])
    outT = nc.dram_tensor("outT", [D, S], MDT, kind="ExternalOutput").ap()
    rcb = nc.dram_tensor("rcb", [2, S], MDT).ap()   # recip bounce (internal)

    Exp = mybir.ActivationFunctionType.Exp

    with tile.TileContext(nc) as tc, ExitStack() as ctx, \
            nc.allow_low_precision(reason="bf16 kernel by design"):
        const = ctx.enter_context(tc.tile_pool(name="const", bufs=1))
        big = ctx.enter_context(tc.tile_pool(name="big", bufs=1))
        stream = ctx.enter_context(tc.tile_pool(name="stream", bufs=1))
        ptile = ctx.enter_context(tc.tile_pool(name="ptile", bufs=1))
        small = ctx.enter_context(tc.tile_pool(name="small", bufs=1))
        outb = ctx.enter_context(tc.tile_pool(name="outb", bufs=1))
        psum = ctx.enter_context(tc.tile_pool(name="psum", bufs=1, space="PSUM"))

        def mm(out, lhsT, rhs, start, stop, tp=None, skip=False):
            nc.tensor.matmul(out, lhsT=lhsT, rhs=rhs, start=start, stop=stop,
                             tile_position=tp, skip_group_check=skip)

        # ---- constants: sync ring gets V/K weights (needed first); q input
        # chunks ride the gpsimd ring; tables/wo on the scalar ring ----
        wv_sb = const.tile([128, NF, 64], MDT)
        nc.sync.dma_start(out=wv_sb, in_=wv.rearrange("p (f d) -> p f d", f=NF))
        wk_sb = const.tile([128, NF, 64], MDT)
        nc.sync.dma_start(out=wk_sb, in_=wk.rearrange("p (f d) -> p f d", f=NF))
        cos_sb = const.tile([128, S], MDT)
        sin_sb = const.tile([128, S], MDT)
        tril_sb = const.tile([128, 128], MDT)
        nc.scalar.dma_start(out=tril_sb, in_=tril)
        ident = const.tile([64, 64], MDT)
        nc.scalar.dma_start(out=ident, in_=ident_in)
        wo_sb = const.tile([128, D], MDT)
        nc.scalar.dma_start(out=wo_sb, in_=wo)
        wq_sb = const.tile([128, NF, 128], MDT)
        bq_sb = const.tile([128, 1], f32)
        bkv_sb = const.tile([128, 1], f32)

        qh = big.tile([128, S], MDT)
        khT2 = big.tile([128, S], MDT)
        vhT = big.tile([64, S], MDT)
        vh_aug = big.tile([128, NJ, 65], MDT)
        attn = big.tile([128, S], MDT)
        nc.vector.memset(vh_aug[:, :, 64], 1.0)

        def oproj(it):
            isl = slice(it * ST, (it + 1) * ST)
            for ep in range(NE // 2):
                pw = psum.tile([128, 2, ST], f32, tag="mm", bufs=2, name="pw")
                for half in range(2):
                    e = 2 * ep + half
                    mm(pw[:, half, :], wo_sb[:, e * 128:(e + 1) * 128],
                       attn[:, isl], start=True, stop=True)
                ob = outb.tile([128, 2, ST], MDT, tag="ob", bufs=3)
                if ep % 2 == 1:
                    nc.scalar.copy(ob, pw)
                else:
                    nc.vector.tensor_copy(ob, pw)
                nc.gpsimd.dma_start(
                    out=outT.rearrange("(g p) s -> p g s", p=128)
                    [:, 2 * ep:2 * ep + 2, isl],
                    in_=ob)

        def proj_rope(s):
            ssl = slice(s * ST, (s + 1) * ST)
            # ---- input streams for this s-tile (contiguous 1MB chunks) ----
            xv = stream.tile([128, NF, ST], MDT, tag="xv", bufs=3)
            nc.sync.dma_start(
                out=xv, in_=vT[s].rearrange("p (f c) -> p f c", f=NF))
            xk = stream.tile([128, NF, ST], MDT, tag="xk", bufs=3)
            nc.sync.dma_start(
                out=xk, in_=kT[s].rearrange("p (f c) -> p f c", f=NF))
            xq = stream.tile([128, NF, ST], MDT, tag="xq", bufs=3)
            nc.gpsimd.dma_start(
                out=xq, in_=qT[s].rearrange("p (f c) -> p f c", f=NF))
            if s == 0:
                # after the first input chunks: rope tables + Q weights
                nc.sync.dma_start(out=cos_sb, in_=cosk)
                nc.sync.dma_start(out=sin_sb, in_=sink)
                nc.sync.dma_start(
                    out=wq_sb, in_=wq.rearrange("p (f d) -> p f d", f=NF))
                nc.sync.dma_start(out=bq_sb, in_=bq)
                nc.sync.dma_start(out=bkv_sb, in_=bkv)

            # ---- V (col grp 0-1) + K (col grp 2-3) projection, col-tiled ----
            ps_kv = psum.tile([128, ST], f32, tag="acc", bufs=4, name="pskv")
            for f in range(NF):
                mm(ps_kv[0:64, :], wv_sb[:, f, :], xv[:, f, :],
                   start=(f == 0), stop=(f == NF - 1), tp=(0, 0))
                mm(ps_kv[64:128, :], wk_sb[:, f, :], xk[:, f, :],
                   start=(f == 0), stop=(f == NF - 1), tp=(0, 64), skip=True)
            nc.vector.tensor_scalar_add(vhT[:, ssl], ps_kv[0:64, :],
                                        bkv_sb[0:64, :])
            nc.vector.tensor_scalar_add(khT2[64:128, ssl], ps_kv[64:128, :],
                                        bkv_sb[64:128, :])

            # ---- Q projection (both heads: 128 out dims) ----
            ps_q = psum.tile([128, ST], f32, tag="acc", bufs=4, name="psq")
            for f in range(NF):
                mm(ps_q, wq_sb[:, f, :], xq[:, f, :],
                   start=(f == 0), stop=(f == NF - 1))
            nc.vector.tensor_scalar_add(qh[:, ssl], ps_q, bq_sb)

            # ---- transpose V to [seq, dim] in vh_aug (ones col = denom) ----
            # before rope: keeps the PE's tp-slot release near the front of
            # the DVE queue so the transposes don't stall behind rope
            for m in range(4):
                jt = 4 * s + m
                tp_ps = psum.tile([128, 2, ST], MDT, tag="mm", bufs=2, name="tp")
                nc.tensor.transpose(tp_ps[:, 0, 0:64],
                                    vhT[:, jt * JTS:(jt + 1) * JTS], ident)
                nc.vector.tensor_copy(vh_aug[:, jt, 0:64], tp_ps[:, 0, 0:64])

            # ---- RoPE (pairs are 32-partition blocks; swap + 2 mul + add) ----
            ksw = ptile.tile([128, ST], MDT, tag="ksw", bufs=2)
            nc.vector.tensor_copy(ksw[64:96, :], khT2[96:128, ssl])
            nc.vector.tensor_copy(ksw[96:128, :], khT2[64:96, ssl])
            nc.vector.tensor_mul(ksw[64:128, :], ksw[64:128, :],
                                 sin_sb[64:128, ssl])
            nc.vector.tensor_mul(khT2[64:128, ssl], khT2[64:128, ssl],
                                 cos_sb[64:128, ssl])
            nc.vector.tensor_add(khT2[64:128, ssl], khT2[64:128, ssl],
                                 ksw[64:128, :])
            # duplicate roped kv head into partitions 0:64 for head-0 scores
            nc.gpsimd.dma_start(out=khT2[0:64, ssl], in_=khT2[64:128, ssl])

            qsw = ptile.tile([128, ST], MDT, tag="qsw", bufs=2)
            for (dstp, srcp) in ((0, 32), (32, 0), (64, 96), (96, 64)):
                nc.vector.tensor_copy(qsw[dstp:dstp + 32, :],
                                      qh[srcp:srcp + 32, ssl])
            nc.vector.tensor_mul(qsw, qsw, sin_sb[:, ssl])
            nc.vector.tensor_mul(qh[:, ssl], qh[:, ssl], cos_sb[:, ssl])
            nc.vector.tensor_add(qh[:, ssl], qh[:, ssl], qsw)

        def attn_block(it):
            # ---- attention for it (both heads, row-tiled scores) ----
            po0 = psum.tile([65, ST], f32, tag="acc", bufs=4, name="po0")
            po1 = psum.tile([65, ST], f32, tag="acc", bufs=4, name="po1")
            jmax = 4 * it + 3
            for jt in range(jmax + 1):
                lo = (jt - 4 * it) * JTS if jt >= 4 * it else 0
                jsl = slice(jt * JTS, (jt + 1) * JTS)
                isl = slice(it * ST + lo, (it + 1) * ST)
                pair = psum.tile([128, 2, ST], f32, tag="mm", bufs=2,
                                 name="pair")
                mm(pair[:, 0, lo:], khT2[0:64, jsl], qh[0:64, isl],
                   start=True, stop=True, tp=(0, 0))
                mm(pair[:, 1, lo:], khT2[64:128, jsl], qh[64:128, isl],
                   start=True, stop=True, tp=(64, 0))
                pt = ptile.tile([128, 2, ST], MDT, tag="pt", bufs=3)
                nc.scalar.activation(out=pt[:, :, lo:], in_=pair[:, :, lo:],
                                     func=Exp, scale=0.125)
                if jt >= 4 * it:
                    for half in range(2):
                        nc.gpsimd.tensor_mul(pt[:, half, lo:lo + JTS],
                                             pt[:, half, lo:lo + JTS], tril_sb)
                if debug and it == 1 and jt == 2:
                    dpt = nc.dram_tensor("d_pt", [128, 2 * ST], MDT,
                                         kind="ExternalOutput").ap()
                    nc.sync.dma_start(
                        out=dpt.rearrange("p (a b) -> p a b", a=2), in_=pt)
                mm(po0[:, lo:], vh_aug[:, jt, :], pt[:, 0, lo:],
                   start=(jt == 0), stop=(jt == jmax))
                mm(po1[:, lo:], vh_aug[:, jt, :], pt[:, 1, lo:],
                   start=(jt == 0), stop=(jt == jmax))

            # ---- softmax denominators -> bf16 reciprocals -> bounce ----
            isl = slice(it * ST, (it + 1) * ST)
            sums = small.tile([1, 2, ST], f32, tag="sums", bufs=2)
            rc = small.tile([1, 2, ST], f32, tag="rc", bufs=2)
            rcb16 = small.tile([1, 2, ST], MDT, tag="rcb16", bufs=2)
            nc.scalar.copy(sums[:, 0, :], po0[64:65, :])
            nc.scalar.copy(sums[:, 1, :], po1[64:65, :])
            nc.vector.reciprocal_approx_fast(rc, sums)
            nc.vector.tensor_copy(rcb16, rc)
            nc.sync.dma_start(out=rcb[0:1, isl], in_=rcb16[:, 0, :])
            nc.sync.dma_start(out=rcb[1:2, isl], in_=rcb16[:, 1, :])
            bct = ptile.tile([128, ST], MDT, tag="bct", bufs=2)
            for h in range(2):
                rsrc = rcb[h:h + 1, isl]
                rsrc = bass.AP(tensor=rsrc.tensor, offset=rsrc.offset,
                               ap=[[0, 64]] + list(rsrc.ap)[1:])
                nc.gpsimd.dma_start(out=bct[h * 64:(h + 1) * 64, :], in_=rsrc)
            if debug and it == 1:
                dbc = nc.dram_tensor("d_bct", [128, ST], MDT,
                                     kind="ExternalOutput").ap()
                nc.sync.dma_start(out=dbc, in_=bct)
            nc.vector.tensor_mul(attn[0:64, isl], po0[0:64, :], bct[0:64, :])
            nc.vector.tensor_mul(attn[64:128, isl], po1[0:64, :],
                                 bct[64:128, :])

        # software pipeline: attention lags projections by one s-tile,
        # output projection lags attention by one more
        for s in range(NS):
            proj_rope(s)
            if s >= 1:
                attn_block(s - 1)
            if s >= 2:
                oproj(s - 2)
        oproj(NS - 2)
        attn_block(NS - 1)
        oproj(NS - 1)

        if debug:
            dqh = nc.dram_tensor("d_qh", [128, S], MDT,
                                 kind="ExternalOutput").ap()
            dkh = nc.dram_tensor("d_khT2", [128, S], MDT,
                                 kind="ExternalOutput").ap()
            dvh = nc.dram_tensor("d_vhT", [64, S], MDT,
                                 kind="ExternalOutput").ap()
            dva = nc.dram_tensor("d_vh_aug", [128, NJ * 65], MDT,
                                 kind="ExternalOutput").ap()
            dat = nc.dram_tensor("d_attn", [128, S], MDT,
                                 kind="ExternalOutput").ap()
            nc.sync.dma_start(out=dqh, in_=qh)
            nc.sync.dma_start(out=dkh, in_=khT2)
            nc.sync.dma_start(out=dvh, in_=vhT)
            nc.sync.dma_start(
                out=dva.rearrange("p (j e) -> p j e", j=NJ), in_=vh_aug)
            nc.sync.dma_start(out=dat, in_=attn)

    nc.compile()
    _CACHE[key] = nc
    return nc


def _host_tables():
    if "tables" in _CACHE:
        return _CACHE["tables"]
    # faithful to reference: exp = -2*arange(0,64,2)/64
    expv = -2.0 * np.arange(0, HD, 2, dtype=np.float32) / HD
    thetas = np.power(np.float32(10000.0), expv).astype(np.float32)    # [32]
    m = np.arange(S, dtype=np.float32)
    freq = np.outer(m, thetas).astype(np.float32)                      # [S, 32]
    cos = np.cos(freq).astype(np.float32).T                            # [32, S]
    sin = np.sin(freq).astype(np.float32).T
    cos128 = np.concatenate([cos, cos, cos, cos], 0)                   # [128, S]
    sin128 = np.concatenate([-sin, sin, -sin, sin], 0)
    perm = np.concatenate([np.arange(0, HD, 2), np.arange(1, HD, 2)])  # deint
    trilm = (np.arange(128)[:, None] <= np.arange(128)[None, :])
    _CACHE["tables"] = (
        np.ascontiguousarray(cos128.astype(NPBF)),
        np.ascontiguousarray(sin128.astype(NPBF)),
        perm,
        np.ascontiguousarray(trilm.astype(NPBF)),
    )
    return _CACHE["tables"]


def _warr(w):
    # [1024, nd] -> [128, NF*nd] with chunk-of-128-rows as middle dim
    nd = w.shape[1]
    return np.ascontiguousarray(
        w.reshape(NF, FP, nd).transpose(1, 0, 2).reshape(FP, NF * nd)
        .astype(NPBF))


def kernel(**inputs):
    q = np.asarray(inputs["q"], np.float32)[0]       # [S, D]
    k = np.asarray(inputs["k"], np.float32)[0]
    v = np.asarray(inputs["v"], np.float32)[0]
    Wq = np.asarray(inputs["Wq"], np.float32)
    Wk = np.asarray(inputs["Wk"], np.float32)
    Wv = np.asarray(inputs["Wv"], np.float32)
    Wo = np.asarray(inputs["Wo"], np.float32)
    bq = np.asarray(inputs["bq"], np.float32)
    bk = np.asarray(inputs["bk"], np.float32)
    bv = np.asarray(inputs["bv"], np.float32)
    bo = np.asarray(inputs["bo"], np.float32)

    cos128, sin128, perm, trilm = _host_tables()

    # head_dim deinterleave permutation applied to q/k projection columns
    permQ = np.concatenate([h * HD + perm for h in range(HEADS)])
    permK = np.concatenate([g * HD + perm for g in range(KVH)])
    Wqp = Wq[:, permQ]
    bqp = bq[permQ]
    Wkp = Wk[:, permK]
    bkp = bk[permK]

    def chunk(x):
        # [S, D] -> [NS, 128, NF*512]: xc[s, p, f*512+c] = x[s*512+c, f*128+p]
        xc = x.T.reshape(NF, FP, NS, ST).transpose(2, 1, 0, 3)
        return np.ascontiguousarray(
            xc.reshape(NS, FP, NF * ST).astype(NPBF))

    qT = chunk(q)
    kT = chunk(k)
    vT = chunk(v)
    ident64 = np.eye(64, dtype=np.float32).astype(NPBF)

    in_maps = []
    for c in range(N_CORES):
        g = c // 2
        if c % 2 == 0:
            h0, h1 = g, g + 4
        else:
            h0, h1 = g + 8, g + 12
        wq_c = np.concatenate([Wqp[:, h0 * HD:(h0 + 1) * HD],
                               Wqp[:, h1 * HD:(h1 + 1) * HD]], axis=1)
        bq_c = np.ascontiguousarray(
            np.concatenate([bqp[h0 * HD:(h0 + 1) * HD],
                            bqp[h1 * HD:(h1 + 1) * HD]]).reshape(128, 1))
        bkv_c = np.ascontiguousarray(
            np.concatenate([bv[g * HD:(g + 1) * HD],
                            bkp[g * HD:(g + 1) * HD]]).reshape(128, 1))
        wo_c = np.ascontiguousarray(
            np.concatenate([Wo[h0 * HD:(h0 + 1) * HD, :],
                            Wo[h1 * HD:(h1 + 1) * HD, :]], axis=0)
            .astype(NPBF))

        in_maps.append({
            "qT": qT, "kT": kT, "vT": vT,
            "wq": _warr(wq_c),
            "wk": _warr(Wkp[:, g * HD:(g + 1) * HD]),
            "wv": _warr(Wv[:, g * HD:(g + 1) * HD]),
            "wo": wo_c,
            "bq": bq_c,
            "bkv": bkv_c,
            "cosk": cos128, "sink": sin128, "tril": trilm,
            "ident": ident64,
        })

    nc = _build_program()
    res = run_bass_kernel_spmd(nc, in_maps, list(range(N_CORES)))
    acc = np.zeros((D, S), np.float32)
    for r in res.results:
        acc += np.asarray(r["outT"], np.float32)
    out = acc.T + bo[None, :]
    return out[None].astype(np.float32)
